# revision 5
# baseline (speedup 1.0000x reference)
"""Batched attention (no-scale softmax) for Trainium2, 8 NeuronCores.

Problem: q [16,2048,128] f32, k [16,128,2048] f32, v [16,2048,128] f32
         out = softmax(q @ k, axis=-1) @ v          -> [16,2048,128] f32

Sharding: batch dim split across 8 cores (2 batches/core), no communication.

Per-core kernel layout ("Layout B" — transposed scores):
  - scores^T [j, i] comes straight out of the PE: lhsT = k-block [d, j],
    rhs = q^T [d, i] (q transposed once per batch on the PE).
  - exp on ACT (PSUM -> SBUF, bf16), no max subtraction (scores ~ N(0,128),
    |s| < ~70, exp stays inside fp32/bf16 range; softmax is shift-invariant
    so this matches the reference up to fp error).
  - PV: lhsT = v-block [j, d] (bf16), rhs = exp^T [j, i] -> out^T [d, i]
    accumulated over j in PSUM (one full bank per 512-wide i chunk).
  - softmax denominators: exp^T chunks accumulated over j-blocks on DVE,
    then one ones-vector matmul reduces the remaining 128 partitions.
  - out^T transposed back to [i, d] on the PE, multiplied by 1/sums
    (per-partition scalar), DMA'd out.
"""

import sys

sys.path.insert(0, "/opt/trn_rl_repo")

import numpy as np

import concourse.bacc as bacc
import concourse.tile as tile
from concourse import mybir
from concourse.bass_utils import run_bass_kernel_spmd
from concourse.masks import make_identity

B, N, D = 16, 2048, 128
N_CORES = 8
BPC = B // N_CORES  # batches per core
NT = N // 128  # 16 blocks of 128 along N
IW = 1024  # i-tile width processed per inner pass (2 passes of 1024)
NIH = N // IW  # 2
NC_PER_IW = IW // 128  # 8 i-blocks per pass

F32 = mybir.dt.float32
F32R = mybir.dt.float32r
BF16 = mybir.dt.bfloat16


def build_nc(qk_f32r=True, probs_bf16=True):
    nc = bacc.Bacc(
        "TRN2", target_bir_lowering=False, debug=False, enable_asserts=False
    )
    q_d = nc.dram_tensor("q", [BPC, N, D], F32, kind="ExternalInput").ap()
    k_d = nc.dram_tensor("k", [BPC, D, N], F32, kind="ExternalInput").ap()
    v_d = nc.dram_tensor("v", [BPC, N, D], F32, kind="ExternalInput").ap()
    o_d = nc.dram_tensor("out", [BPC, N, D], F32, kind="ExternalOutput").ap()

    PDT = BF16 if probs_bf16 else F32
    QKDT = F32R if qk_f32r else F32

    with tile.TileContext(nc) as tc:
        with (
            tc.tile_pool(name="consts", bufs=1) as consts,
            tc.tile_pool(name="kp", bufs=2) as kp,
            tc.tile_pool(name="qp", bufs=2) as qp,
            tc.tile_pool(name="qtp", bufs=2) as qtp,
            tc.tile_pool(name="vfp", bufs=2) as vfp,
            tc.tile_pool(name="vbp", bufs=2) as vbp,
            tc.tile_pool(name="etp", bufs=3) as etp,
            tc.tile_pool(name="accp", bufs=2) as accp,
            tc.tile_pool(name="otsp", bufs=4) as otsp,
            tc.tile_pool(name="osp", bufs=4) as osp,
            tc.tile_pool(name="rsp", bufs=2) as rsp,
            tc.tile_pool(name="dramp", bufs=2, space="DRAM") as dramp,
            tc.tile_pool(name="stp", bufs=2, space="PSUM") as stp,
            tc.tile_pool(name="otp", bufs=2, space="PSUM") as otp,
            tc.tile_pool(name="o2p", bufs=2, space="PSUM") as o2p,
        ):
            identity = consts.tile([128, 128], F32)
            make_identity(nc, identity)
            ones = consts.tile([128, 1], F32)
            nc.vector.memset(ones, 1.0)

            for b in range(BPC):
                # ---- loads ----
                k_sb = kp.tile([128, N], QKDT, tag="k")
                nc.sync.dma_start(out=k_sb, in_=k_d[b].bitcast(QKDT))
                q_sb = qp.tile([128, NT, 128], F32, tag="q")
                nc.sync.dma_start(
                    out=q_sb, in_=q_d[b].rearrange("(t p) d -> p t d", p=128)
                )
                vf_sb = vfp.tile([128, NT, 128], F32, tag="vf")
                nc.sync.dma_start(
                    out=vf_sb, in_=v_d[b].rearrange("(t p) d -> p t d", p=128)
                )
                v_bf = vbp.tile([128, NT, 128], PDT, tag="vb")
                nc.vector.tensor_copy(out=v_bf, in_=vf_sb)

                # ---- q^T via PE transposes ----
                qT_sb = qtp.tile([128, N], QKDT, tag="qt")
                for t in range(NT):
                    qt_ps = stp.tile([128, 128], F32, tag="st")
                    nc.tensor.transpose(qt_ps, q_sb[:, t, :], identity)
                    nc.vector.tensor_copy(
                        out=qT_sb[:, t * 128 : (t + 1) * 128], in_=qt_ps
                    )

                for ih in range(NIH):
                    i0 = ih * IW
                    # out^T accumulators: one PSUM bank per 512-wide i chunk
                    outT = [
                        otp.tile([128, 512], F32, tag="ot", name="outT")
                        for _ in range(IW // 512)
                    ]
                    acc = accp.tile([128, IW], F32, tag="acc")
                    for jb in range(NT):
                        st = stp.tile([128, IW], F32, tag="st")
                        for c in range(IW // 512):
                            nc.tensor.matmul(
                                st[:, c * 512 : (c + 1) * 512],
                                lhsT=k_sb[:, jb * 128 : (jb + 1) * 128],
                                rhs=qT_sb[:, i0 + c * 512 : i0 + (c + 1) * 512],
                                start=True,
                                stop=True,
                            )
                        et = etp.tile([128, IW], PDT, tag="et")
                        nc.scalar.activation(
                            out=et, in_=st, func=mybir.ActivationFunctionType.Exp
                        )
                        for c in range(IW // 512):
                            nc.tensor.matmul(
                                outT[c],
                                lhsT=v_bf[:, jb, :],
                                rhs=et[:, c * 512 : (c + 1) * 512],
                                start=(jb == 0),
                                stop=(jb == NT - 1),
                            )
                        if jb == 0:
                            nc.vector.tensor_copy(out=acc, in_=et)
                        else:
                            nc.vector.tensor_add(out=acc, in0=acc, in1=et)

                    # ---- softmax denominators for this i range ----
                    rs_sb = rsp.tile([1, IW], F32, tag="rs")
                    for c in range(IW // 512):
                        sums = otp.tile([1, 512], F32, tag="ot")
                        nc.tensor.matmul(
                            sums,
                            lhsT=ones,
                            rhs=acc[:, c * 512 : (c + 1) * 512],
                            start=True,
                            stop=True,
                        )
                        nc.vector.reciprocal(
                            out=rs_sb[:, c * 512 : (c + 1) * 512], in_=sums
                        )
                    # scatter 1/sums across partitions: [1, IW] -> [128, IW/128]
                    rs_dram = dramp.tile([IW], F32, tag="rsd")
                    nc.sync.dma_start(
                        out=rs_dram.rearrange("(o n) -> o n", o=1), in_=rs_sb
                    )
                    rs_t = rsp.tile([128, NC_PER_IW], F32, tag="rst")
                    nc.sync.dma_start(
                        out=rs_t, in_=rs_dram.rearrange("(t p) -> p t", p=128)
                    )

                    # ---- finalize: transpose out^T back, normalize, store ----
                    oT_sb = []
                    for c in range(IW // 512):
                        o_c = otsp.tile([128, 512], F32, tag="ots")
                        nc.vector.tensor_copy(out=o_c, in_=outT[c])
                        oT_sb.append(o_c)
                    for t8 in range(NC_PER_IW):
                        c, off = t8 // 4, (t8 % 4) * 128
                        out2 = o2p.tile([128, 128], F32, tag="o2")
                        nc.tensor.transpose(
                            out2, oT_sb[c][:, off : off + 128], identity
                        )
                        out_sb = osp.tile([128, 128], F32, tag="os")
                        nc.vector.tensor_scalar_mul(
                            out_sb, out2, rs_t[:, t8 : t8 + 1]
                        )
                        t = ih * NC_PER_IW + t8
                        nc.sync.dma_start(
                            out=o_d[b, t * 128 : (t + 1) * 128, :], in_=out_sb
                        )

    nc.compile()
    return nc


_NC_CACHE = {}


def _get_nc(key=(True, True)):
    if key not in _NC_CACHE:
        _NC_CACHE[key] = build_nc(*key)
    return _NC_CACHE[key]


def kernel(q, k, v):
    q = np.ascontiguousarray(np.asarray(q), dtype=np.float32)
    k = np.ascontiguousarray(np.asarray(k), dtype=np.float32)
    v = np.ascontiguousarray(np.asarray(v), dtype=np.float32)
    nc = _get_nc()
    in_maps = [
        {
            "q": q[c * BPC : (c + 1) * BPC],
            "k": k[c * BPC : (c + 1) * BPC],
            "v": v[c * BPC : (c + 1) * BPC],
        }
        for c in range(N_CORES)
    ]
    res = run_bass_kernel_spmd(nc, in_maps, core_ids=list(range(N_CORES)))
    return np.concatenate([res.results[c]["out"] for c in range(N_CORES)], axis=0)


# revision 10
# speedup vs baseline: 6264.0097x; 6264.0097x over previous
"""Batched attention (no-scale softmax) for Trainium2, 8 NeuronCores.

Problem: q [16,2048,128] f32, k [16,128,2048] f32, v [16,2048,128] f32
         out = softmax(q @ k, axis=-1) @ v          -> [16,2048,128] f32

Sharding: batch dim split across 8 cores (2 batches/core), no communication.

Per-core kernel layout ("Layout B" — transposed scores):
  - scores^T [j, i] comes straight out of the PE: lhsT = k-block [d, j],
    rhs = q^T [d, i] (q transposed once per batch on the PE).
  - exp on ACT (PSUM -> SBUF, bf16), no max subtraction (scores ~ N(0,128),
    |s| < ~70, exp stays inside fp32/bf16 range; softmax is shift-invariant
    so this matches the reference up to fp error).
  - PV: lhsT = v-block [j, d] (bf16), rhs = exp^T [j, i] -> out^T [d, i]
    accumulated over j in PSUM (one full bank per 512-wide i chunk).
  - softmax denominators: exp^T chunks accumulated over j-blocks on DVE,
    then one ones-vector matmul reduces the remaining 128 partitions.
  - out^T transposed back to [i, d] on the PE, multiplied by 1/sums
    (per-partition scalar), DMA'd out.
"""

import sys

sys.path.insert(0, "/opt/trn_rl_repo")

import numpy as np

import concourse.bacc as bacc
import concourse.tile as tile
from concourse import mybir
from concourse.bass_utils import run_bass_kernel_spmd
from concourse.masks import make_identity

B, N, D = 16, 2048, 128
N_CORES = 8
BPC = B // N_CORES  # batches per core
NT = N // 128  # 16 blocks of 128 along N
IW = 1024  # i-tile width processed per inner pass (2 passes of 1024)
NIH = N // IW  # 2
NC_PER_IW = IW // 128  # 8 i-blocks per pass

F32 = mybir.dt.float32
F32R = mybir.dt.float32r
BF16 = mybir.dt.bfloat16


def build_nc(qk_f32r=True, probs_bf16=True, repeat=1, st_bufs=2, et_bufs=3, o2_bufs=2):
    nc = bacc.Bacc(
        "TRN2", target_bir_lowering=False, debug=False, enable_asserts=False
    )
    q_d = nc.dram_tensor("q", [BPC, N, D], F32, kind="ExternalInput").ap()
    k_d = nc.dram_tensor("k", [BPC, D, N], F32, kind="ExternalInput").ap()
    v_d = nc.dram_tensor("v", [BPC, N, D], F32, kind="ExternalInput").ap()
    o_d = nc.dram_tensor("out", [BPC, N, D], F32, kind="ExternalOutput").ap()

    PDT = BF16 if probs_bf16 else F32
    QKDT = F32R if qk_f32r else F32

    with tile.TileContext(nc) as tc:
        with (
            tc.tile_pool(name="consts", bufs=1) as consts,
            tc.tile_pool(name="kp", bufs=2) as kp,
            tc.tile_pool(name="qp", bufs=2) as qp,
            tc.tile_pool(name="qtp", bufs=2) as qtp,
            tc.tile_pool(name="vfp", bufs=2) as vfp,
            tc.tile_pool(name="vbp", bufs=2) as vbp,
            tc.tile_pool(name="etp", bufs=et_bufs) as etp,
            tc.tile_pool(name="accp", bufs=2) as accp,
            tc.tile_pool(name="otsp", bufs=4) as otsp,
            tc.tile_pool(name="osp", bufs=4) as osp,
            tc.tile_pool(name="rsp", bufs=2) as rsp,
            tc.tile_pool(name="dramp", bufs=2, space="DRAM") as dramp,
            tc.tile_pool(name="stp", bufs=st_bufs, space="PSUM") as stp,
            tc.tile_pool(name="otp", bufs=2, space="PSUM") as otp,
            tc.tile_pool(name="o2p", bufs=o2_bufs, space="PSUM") as o2p,
        ):
            identity = consts.tile([128, 128], F32)
            make_identity(nc, identity)
            ones = consts.tile([128, 1], F32)
            nc.vector.memset(ones, 1.0)

            for b in [b for _ in range(repeat) for b in range(BPC)]:
                # ---- loads ----
                k_sb = kp.tile([128, N], QKDT, tag="k")
                nc.sync.dma_start(out=k_sb, in_=k_d[b].bitcast(QKDT))
                q_sb = qp.tile([128, NT, 128], F32, tag="q")
                nc.sync.dma_start(
                    out=q_sb, in_=q_d[b].rearrange("(t p) d -> p t d", p=128)
                )
                vf_sb = vfp.tile([128, NT, 128], F32, tag="vf")
                nc.sync.dma_start(
                    out=vf_sb, in_=v_d[b].rearrange("(t p) d -> p t d", p=128)
                )
                v_bf = vbp.tile([128, NT, 128], PDT, tag="vb")
                nc.vector.tensor_copy(out=v_bf, in_=vf_sb)

                # ---- q^T via PE transposes ----
                qT_sb = qtp.tile([128, N], QKDT, tag="qt")
                for t in range(NT):
                    qt_ps = stp.tile([128, 128], F32, tag="st")
                    nc.tensor.transpose(qt_ps, q_sb[:, t, :], identity)
                    nc.vector.tensor_copy(
                        out=qT_sb[:, t * 128 : (t + 1) * 128], in_=qt_ps
                    )

                for ih in range(NIH):
                    i0 = ih * IW
                    # out^T accumulators: one PSUM bank per 512-wide i chunk
                    outT = [
                        otp.tile([128, 512], F32, tag="ot", name="outT")
                        for _ in range(IW // 512)
                    ]
                    acc = accp.tile([128, IW], F32, tag="acc")
                    for jb in range(NT):
                        st = stp.tile([128, IW], F32, tag="st")
                        for c in range(IW // 512):
                            nc.tensor.matmul(
                                st[:, c * 512 : (c + 1) * 512],
                                lhsT=k_sb[:, jb * 128 : (jb + 1) * 128],
                                rhs=qT_sb[:, i0 + c * 512 : i0 + (c + 1) * 512],
                                start=True,
                                stop=True,
                            )
                        et = etp.tile([128, IW], PDT, tag="et")
                        nc.scalar.activation(
                            out=et, in_=st, func=mybir.ActivationFunctionType.Exp
                        )
                        for c in range(IW // 512):
                            nc.tensor.matmul(
                                outT[c],
                                lhsT=v_bf[:, jb, :],
                                rhs=et[:, c * 512 : (c + 1) * 512],
                                start=(jb == 0),
                                stop=(jb == NT - 1),
                            )
                        if jb == 0:
                            nc.vector.tensor_copy(out=acc, in_=et)
                        else:
                            nc.vector.tensor_add(out=acc, in0=acc, in1=et)

                    # ---- softmax denominators for this i range ----
                    rs_sb = rsp.tile([1, IW], F32, tag="rs")
                    for c in range(IW // 512):
                        sums = otp.tile([1, 512], F32, tag="ot")
                        nc.tensor.matmul(
                            sums,
                            lhsT=ones,
                            rhs=acc[:, c * 512 : (c + 1) * 512],
                            start=True,
                            stop=True,
                        )
                        nc.vector.reciprocal(
                            out=rs_sb[:, c * 512 : (c + 1) * 512], in_=sums
                        )
                    # scatter 1/sums across partitions: [1, IW] -> [128, IW/128]
                    rs_dram = dramp.tile([IW], F32, tag="rsd")
                    nc.sync.dma_start(
                        out=rs_dram.rearrange("(o n) -> o n", o=1), in_=rs_sb
                    )
                    rs_t = rsp.tile([128, NC_PER_IW], F32, tag="rst")
                    nc.sync.dma_start(
                        out=rs_t, in_=rs_dram.rearrange("(t p) -> p t", p=128)
                    )

                    # ---- finalize: transpose out^T back, normalize, store ----
                    oT_sb = []
                    for c in range(IW // 512):
                        o_c = otsp.tile([128, 512], F32, tag="ots")
                        nc.vector.tensor_copy(out=o_c, in_=outT[c])
                        oT_sb.append(o_c)
                    for t8 in range(NC_PER_IW):
                        c, off = t8 // 4, (t8 % 4) * 128
                        out2 = o2p.tile([128, 128], F32, tag="o2")
                        nc.tensor.transpose(
                            out2, oT_sb[c][:, off : off + 128], identity
                        )
                        out_sb = osp.tile([128, 128], F32, tag="os")
                        nc.vector.tensor_scalar_mul(
                            out_sb, out2, rs_t[:, t8 : t8 + 1]
                        )
                        t = ih * NC_PER_IW + t8
                        nc.sync.dma_start(
                            out=o_d[b, t * 128 : (t + 1) * 128, :], in_=out_sb
                        )

    nc.compile()
    return nc


_NC_CACHE = {}


def _get_nc(key=(True, True)):
    if key not in _NC_CACHE:
        _NC_CACHE[key] = build_nc(*key)
    return _NC_CACHE[key]


def kernel(q, k, v):
    q = np.ascontiguousarray(np.asarray(q), dtype=np.float32)
    k = np.ascontiguousarray(np.asarray(k), dtype=np.float32)
    v = np.ascontiguousarray(np.asarray(v), dtype=np.float32)
    nc = _get_nc()
    in_maps = [
        {
            "q": q[c * BPC : (c + 1) * BPC],
            "k": k[c * BPC : (c + 1) * BPC],
            "v": v[c * BPC : (c + 1) * BPC],
        }
        for c in range(N_CORES)
    ]
    res = run_bass_kernel_spmd(nc, in_maps, core_ids=list(range(N_CORES)))
    return np.concatenate([res.results[c]["out"] for c in range(N_CORES)], axis=0)
